# revision 7
# baseline (speedup 1.0000x reference)
"""BPS-DenseNet Trainium2 kernel.

kernel(**inputs) -> [32, 512] f32. Shards the BPS distance computation
data-parallel over batch across 8 NeuronCores, AllGathers the BPS features,
then every core computes the (tiny) BN-MLP head redundantly; core 0's output
is returned.
"""
import os
import sys
import types

sys.path.insert(0, '/opt/trn_rl_repo')
import numpy as np

# --- optional NTFF profile hook (only when BPS_TRACE=1; grading path skips) ---
TRACE = os.environ.get("BPS_TRACE", "0") == "1"
ABLATE = int(os.environ.get("BPS_ABLATE", "0"))
if TRACE:
    import antenv
    _mod = types.ModuleType("antenv.axon_hooks")
    _mod._hook = None
    _mod.set_axon_ntff_profile_hook = lambda h: setattr(_mod, "_hook", h)
    _mod.get_axon_ntff_profile_hook = lambda: _mod._hook
    sys.modules["antenv.axon_hooks"] = _mod
    antenv.axon_hooks = _mod
    from trn_agent_boot.trn_boot import _ntff_profile_via_ctypes
    _mod._hook = _ntff_profile_via_ctypes('/opt/axon/libaxon_pjrt.so')

import concourse.bacc as bacc
import concourse.mybir as mybir
import concourse.tile as tile
from concourse import bass_utils

bass_utils.upload_artifacts = lambda tmpdir: tmpdir

B, N, P, H, E = 32, 2048, 1024, 256, 512
NC = 8
BL = B // NC            # batches per core
MCH = P // 128          # basis chunks
EPS = 1e-5

F32 = mybir.dt.float32
F16 = mybir.dt.float16

# layer defs: (name, K-tile count, H_out)
# K-tile sources resolved at emission time.
NKT = {"L0": 8, "L1": 2, "L2": 10, "L3": 2, "LF": 12}
HOUT = {"L0": H, "L1": H, "L2": H, "L3": H, "LF": E}
WCOLS = sum(NKT[l] * HOUT[l] for l in ("L0", "L1", "L2", "L3", "LF"))
BIAS_COLS = 4 * H + E

_CACHE = {}


def _build_module():
    nc = bacc.Bacc("TRN2", target_bir_lowering=False, debug=False,
                   num_devices=NC)

    pts5c_d = nc.dram_tensor("pts5c", [5 * BL, N], F32, kind="ExternalInput")
    basis5_d = nc.dram_tensor("basis5", [5, P], F32, kind="ExternalInput")
    wts_d = nc.dram_tensor("wts", [128, WCOLS], F16, kind="ExternalInput")
    bias_d = nc.dram_tensor("biases", [1, BIAS_COLS], F16, kind="ExternalInput")
    gpk_d = nc.dram_tensor("gpk", [128, 20], F32, kind="ExternalInput")
    bpk_d = nc.dram_tensor("bpk", [128, 20], F32, kind="ExternalInput")
    outT_d = nc.dram_tensor("outT", [E, B], F32, kind="ExternalOutput")

    cc0_in = nc.dram_tensor("cc0_in", [1, 4], F32)
    cc0_out = nc.dram_tensor("cc0_out", [NC, 4], F32, addr_space="Shared")
    cc_in = nc.dram_tensor("cc_in", [P, BL], F32)
    cc_out = nc.dram_tensor("cc_out", [NC * P, BL], F32, addr_space="Shared")

    with tile.TileContext(nc) as tc:
        with tc.tile_pool(name="sb", bufs=1) as sb:
            # ---- warm up the collectives subsystem ASAP (hides ~40us) ----
            dummy = sb.tile([1, 4], F32)
            nc.gpsimd.memset(dummy[:], 0.0)
            if ABLATE != 2:
                nc.sync.dma_start(cc0_in[:, :], dummy[:])
                nc.gpsimd.collective_compute(
                    "AllGather", mybir.AluOpType.bypass,
                    replica_groups=[list(range(NC))],
                    ins=[cc0_in.ap().opt()], outs=[cc0_out.ap().opt()])

            # ---- inputs to SBUF ----
            pts5rt = sb.tile([128, N], F32)
            basis5rt = sb.tile([128, P], F32)
            for j in range(BL):
                nc.sync.dma_start(pts5rt[32 * j:32 * j + 5, :],
                                  pts5c_d[5 * j:5 * j + 5, :])
                nc.sync.dma_start(basis5rt[32 * j:32 * j + 5, :],
                                  basis5_d[:, :])
            wts = sb.tile([128, WCOLS], F16)
            nc.sync.dma_start(wts[:], wts_d[:])
            biases = sb.tile([1, BIAS_COLS], F16)
            nc.sync.dma_start(biases[:], bias_d[:])
            gpk = sb.tile([128, 20], F32)
            bpk = sb.tile([128, 20], F32)
            nc.sync.dma_start(gpk[:], gpk_d[:])
            nc.sync.dma_start(bpk[:], bpk_d[:])
            ones_h = sb.tile([1, B], F16)
            nc.gpsimd.memset(ones_h[:], 1.0)

            featA = sb.tile([128, MCH, BL], F32)
            featB = sb.tile([128, MCH, BL], F32)

            # ---- BPS: distances + min-reduce ----
            if ABLATE == 3:
                nc.gpsimd.memset(featA[:], 1.0)
                nc.gpsimd.memset(featB[:], 1.0)
            with tc.tile_pool(name="psb", bufs=1, space="PSUM") as psb:
              if ABLATE != 3:
                  for m in range(MCH):
                      for hh in range(2):
                          for j in range(BL):
                            dps = psb.tile([128, 2, 512], F32, tag=f"d{j}",
                                           name=f"d{m}_{hh}_{j}")
                            for t2 in range(2):
                                t = 2 * hh + t2
                                nc.tensor.matmul(
                                    dps[:, t2, :],
                                    basis5rt[32 * j:32 * j + 5,
                                             m * 128:(m + 1) * 128],
                                    pts5rt[32 * j:32 * j + 5,
                                           t * 512:(t + 1) * 512],
                                    start=True, stop=True,
                                    tile_position=(32 * j, 0))
                            dst = featA if hh == 0 else featB
                            nc.vector.tensor_reduce(
                                dst[:, m, j:j + 1], dps[:],
                                axis=mybir.AxisListType.XY,
                                op=mybir.AluOpType.min)

            # combine halves, clamp, sqrt
            nc.vector.tensor_tensor(featA[:, :, :], featA[:, :, :],
                                    featB[:, :, :], mybir.AluOpType.min)
            nc.vector.tensor_scalar_max(featA[:, :, :], featA[:, :, :], 0.0)
            nc.scalar.activation(featA[:, :, :], featA[:, :, :],
                                 mybir.ActivationFunctionType.Sqrt)

            # ---- AllGather feat shards ----
            feat = sb.tile([128, MCH, NC, BL], F32)
            if ABLATE != 2:
                nc.sync.dma_start(
                    cc_in.ap().rearrange("(m p) b -> p m b", p=128),
                    featA[:, :, :])
                nc.gpsimd.collective_compute(
                    "AllGather", mybir.AluOpType.bypass,
                    replica_groups=[list(range(NC))],
                    ins=[cc_in.ap().opt()], outs=[cc_out.ap().opt()])
                cc_out_r = cc_out.ap().rearrange("(r m p) b -> p m r b",
                                                 p=128, m=MCH)
                for m in range(MCH):
                    nc.sync.dma_start(feat[:, m, :, :], cc_out_r[:, m, :, :])
            else:
                for r in range(NC):
                    nc.vector.tensor_copy(feat[:, :, r, :], featA[:, :, :])

            # ================= MLP head (feature-major, batch on free) ======
            # stat columns: bn0:0-7, L0:8-9, L1:10-11, L2:12-13, L3:14-15, LF:16-19
            SCOL = {"bn0": 0, "L0": 8, "L1": 10, "L2": 12, "L3": 14, "LF": 16}
            NT_L = {"bn0": 8, "L0": 2, "L1": 2, "L2": 2, "L3": 2, "LF": 4}
            sums = sb.tile([128, 20], F32)
            sqs = sb.tile([128, 20], F32)
            scr = sb.tile([128, B], F32)
            scr16 = sb.tile([128, B], F16)

            x0 = sb.tile([128, 8, B], F16)
            h1 = sb.tile([128, 2, B], F16)
            a1 = sb.tile([128, 2, B], F16)
            h2 = sb.tile([128, 2, B], F16)
            a2 = sb.tile([128, 2, B], F16)
            outT = sb.tile([128, 4, B], F32)

            def stats(src_ap, lname, i, fp16):
                c = SCOL[lname] + i
                nc.vector.tensor_reduce(
                    sums[:, c:c + 1], src_ap, axis=mybir.AxisListType.X,
                    op=mybir.AluOpType.add)
                sc = (scr16 if fp16 else scr)[:, :]
                nc.vector.tensor_tensor(sc, src_ap, src_ap,
                                        mybir.AluOpType.mult)
                nc.vector.tensor_reduce(
                    sqs[:, c:c + 1], sc, axis=mybir.AxisListType.X,
                    op=mybir.AluOpType.add)

            def bn_coeffs(lname):
                """Turn sums/sqs cols into A (scale) and C (shift) in-place:
                A -> sums cols, C -> sqs cols."""
                c0 = SCOL[lname]
                c1 = c0 + NT_L[lname]
                mean = scr  # reuse scratch [128, B]; only first cols used
                nc.vector.tensor_scalar_mul(mean[:, c0:c1], sums[:, c0:c1],
                                            1.0 / B)
                nc.vector.tensor_scalar_mul(sqs[:, c0:c1], sqs[:, c0:c1],
                                            1.0 / B)
                # var = E[x^2] - mean^2   (into sqs)
                nc.vector.tensor_tensor(scr[:, c1:c1 + NT_L[lname]],
                                        mean[:, c0:c1], mean[:, c0:c1],
                                        mybir.AluOpType.mult)
                nc.vector.tensor_tensor(sqs[:, c0:c1], sqs[:, c0:c1],
                                        scr[:, c1:c1 + NT_L[lname]],
                                        mybir.AluOpType.subtract)
                nc.vector.tensor_scalar_add(sqs[:, c0:c1], sqs[:, c0:c1], EPS)
                nc.vector.reciprocal(sqs[:, c0:c1], sqs[:, c0:c1])
                nc.scalar.activation(sqs[:, c0:c1], sqs[:, c0:c1],
                                     mybir.ActivationFunctionType.Sqrt)
                # A = g * rsqrt  (into sums)
                nc.vector.tensor_tensor(sums[:, c0:c1], gpk[:, c0:c1],
                                        sqs[:, c0:c1], mybir.AluOpType.mult)
                # C = beta - mean * A  (into sqs)
                nc.vector.tensor_tensor(scr[:, c1:c1 + NT_L[lname]],
                                        mean[:, c0:c1], sums[:, c0:c1],
                                        mybir.AluOpType.mult)
                nc.vector.tensor_tensor(sqs[:, c0:c1], bpk[:, c0:c1],
                                        scr[:, c1:c1 + NT_L[lname]],
                                        mybir.AluOpType.subtract)

            def bn_apply(src_ap, dst_ap, lname, i):
                c = SCOL[lname] + i
                nc.vector.tensor_scalar(
                    out=dst_ap, in0=src_ap,
                    scalar1=sums[:, c:c + 1], scalar2=sqs[:, c:c + 1],
                    op0=mybir.AluOpType.mult, op1=mybir.AluOpType.add)

            # ---- bn0 over feat ----
            for m in range(MCH):
                stats(feat[:, m, :, :].rearrange("p r b -> p (r b)"),
                      "bn0", m, False)
            bn_coeffs("bn0")
            for m in range(MCH):
                bn_apply(feat[:, m, :, :].rearrange("p r b -> p (r b)"),
                         x0[:, m, :], "bn0", m)

            wts_off = [0]

            def layer(lname, ktiles, dst, fp32_out=False):
                """ktiles: list of [128, B] fp16 APs (K-tiles of input).
                dst: output tile [128, NT, B]."""
                hout = HOUT[lname]
                nmo = hout // 128
                base = wts_off[0]
                bbase = {"L0": 0, "L1": H, "L2": 2 * H, "L3": 3 * H,
                         "LF": 4 * H}[lname]
                with tc.tile_pool(name=f"pm{lname}", bufs=4,
                                  space="PSUM") as psm:
                    for mo in range(nmo):
                        zp = psm.tile([128, B], F32, tag="z",
                                      name=f"z{lname}_{mo}")
                        nk = len(ktiles)
                        for k, rhs in enumerate(ktiles):
                            lhsT = wts[:, base + k * hout + mo * 128:
                                       base + k * hout + (mo + 1) * 128]
                            nc.tensor.matmul(zp[:, :], lhsT, rhs,
                                             start=(k == 0), stop=False)
                        nc.tensor.matmul(
                            zp[:, :],
                            biases[0:1, bbase + mo * 128:bbase + (mo + 1) * 128],
                            ones_h[0:1, :], start=False, stop=True)
                        # relu -> fp16 (or fp32 for final)
                        if fp32_out:
                            nc.scalar.activation(
                                dst[:, mo, :], zp[:, :],
                                mybir.ActivationFunctionType.Relu)
                            stats(dst[:, mo, :], lname, mo, False)
                        else:
                            nc.scalar.activation(
                                dst[:, mo, :], zp[:, :],
                                mybir.ActivationFunctionType.Relu)
                            stats(dst[:, mo, :], lname, mo, True)
                wts_off[0] = base + NKT[lname] * hout
                bn_coeffs(lname)
                for mo in range(nmo):
                    bn_apply(dst[:, mo, :], dst[:, mo, :], lname, mo)

            if ABLATE == 1:
                nc.vector.memset(outT[:], 0.25)
            else:
                x0k = [x0[:, m, :] for m in range(8)]
                layer("L0", x0k, h1)
                layer("L1", [h1[:, i, :] for i in range(2)], a1)
                layer("L2", x0k + [a1[:, i, :] for i in range(2)], h2)
                layer("L3", [h2[:, i, :] for i in range(2)], a2)
                layer("LF", x0k + [a1[:, i, :] for i in range(2)]
                      + [a2[:, i, :] for i in range(2)], outT, fp32_out=True)

            outT_r = outT_d.ap().rearrange("(mo p) b -> p mo b", p=128)
            nc.sync.dma_start(outT_r[:, :, :], outT[:, :, :])

    nc.compile()
    return nc


def _prep_inputs(x, basis, bn0_g, bn0_b, W0, b0, g0, beta0, W1, b1, g1, beta1,
                 W2, b2, g2, beta2, W3, b3, g3, beta3, Wf, bf, gf, betaf):
    f32 = np.float32
    f16 = np.float16
    x = np.asarray(x, f32)
    s = (x.astype(np.float64) ** 2).sum(1).astype(f32)        # [B, N]
    basis = np.asarray(basis, f32)

    basis5 = np.zeros((5, P), f32)
    basis5[0:3] = -2.0 * basis.T
    basis5[3] = 1.0
    basis5[4] = (basis ** 2).sum(1)

    def ktile_cols(WT, hout):
        # WT: [K, hout] -> [128, nk*hout] column-packed K-tiles
        nk = WT.shape[0] // 128
        return np.concatenate([WT[k * 128:(k + 1) * 128, :]
                               for k in range(nk)], axis=1)

    wts = np.concatenate([
        ktile_cols(np.ascontiguousarray(W0.T), H),
        ktile_cols(np.ascontiguousarray(W1.T), H),
        ktile_cols(np.ascontiguousarray(W2.T), H),
        ktile_cols(np.ascontiguousarray(W3.T), H),
        ktile_cols(np.ascontiguousarray(Wf.T), E),
    ], axis=1).astype(f16)

    biases = np.concatenate([b0, b1, b2, b3, bf]).reshape(1, -1).astype(f16)

    def pk(v, n):
        return np.asarray(v, f32).reshape(n, 128).T

    gpk = np.concatenate([pk(bn0_g, 8), pk(g0, 2), pk(g1, 2), pk(g2, 2),
                          pk(g3, 2), pk(gf, 4)], axis=1)
    bpk = np.concatenate([pk(bn0_b, 8), pk(beta0, 2), pk(beta1, 2),
                          pk(beta2, 2), pk(beta3, 2), pk(betaf, 4)], axis=1)

    in_maps = []
    for c in range(NC):
        pts5c = np.zeros((5 * BL, N), f32)
        for j in range(BL):
            b = c * BL + j
            pts5c[5 * j:5 * j + 3] = x[b]
            pts5c[5 * j + 3] = s[b]
            pts5c[5 * j + 4] = 1.0
        in_maps.append({"pts5c": pts5c, "basis5": basis5, "wts": wts,
                        "biases": biases, "gpk": gpk, "bpk": bpk})
    return in_maps


LAST_EXEC_NS = None
LAST_PROFILE = None


def kernel(**inputs) -> np.ndarray:
    global LAST_EXEC_NS, LAST_PROFILE
    if "nc" not in _CACHE:
        _CACHE["nc"] = _build_module()
    nc = _CACHE["nc"]
    in_maps = _prep_inputs(**inputs)
    res = bass_utils.run_bass_kernel_spmd(
        nc, in_maps, core_ids=list(range(NC)), trace=TRACE)
    LAST_EXEC_NS = res.exec_time_ns
    LAST_PROFILE = res.profile_json
    outT = res.results[0]["outT"]          # [E, B]
    return np.ascontiguousarray(outT.T)    # [B, E]


# revision 9
# speedup vs baseline: 1.1569x; 1.1569x over previous
"""BPS-DenseNet Trainium2 kernel.

kernel(**inputs) -> [32, 512] f32. Shards the BPS distance computation
data-parallel over batch across 8 NeuronCores, AllGathers the BPS features,
then every core computes the (tiny) BN-MLP head redundantly; core 0's output
is returned.
"""
import os
import sys
import types

sys.path.insert(0, '/opt/trn_rl_repo')
import numpy as np

# --- optional NTFF profile hook (only when BPS_TRACE=1; grading path skips) ---
TRACE = os.environ.get("BPS_TRACE", "0") == "1"
ABLATE = int(os.environ.get("BPS_ABLATE", "0"))
if TRACE:
    import antenv
    _mod = types.ModuleType("antenv.axon_hooks")
    _mod._hook = None
    _mod.set_axon_ntff_profile_hook = lambda h: setattr(_mod, "_hook", h)
    _mod.get_axon_ntff_profile_hook = lambda: _mod._hook
    sys.modules["antenv.axon_hooks"] = _mod
    antenv.axon_hooks = _mod
    from trn_agent_boot.trn_boot import _ntff_profile_via_ctypes
    _mod._hook = _ntff_profile_via_ctypes('/opt/axon/libaxon_pjrt.so')

import concourse.bacc as bacc
import concourse.mybir as mybir
import concourse.tile as tile
from concourse import bass_utils

bass_utils.upload_artifacts = lambda tmpdir: tmpdir

B, N, P, H, E = 32, 2048, 1024, 256, 512
NC = 8
BL = B // NC            # batches per core
MCH = P // 128          # basis chunks
EPS = 1e-5

F32 = mybir.dt.float32
F16 = mybir.dt.float16

# layer defs: (name, K-tile count, H_out)
# K-tile sources resolved at emission time.
NKT = {"L0": 8, "L1": 2, "L2": 10, "L3": 2, "LF": 12}
HOUT = {"L0": H, "L1": H, "L2": H, "L3": H, "LF": E}
WCOLS = sum(NKT[l] * HOUT[l] for l in ("L0", "L1", "L2", "L3", "LF"))
BIAS_COLS = 4 * H + E

_CACHE = {}


def _build_module():
    nc = bacc.Bacc("TRN2", target_bir_lowering=False, debug=False,
                   num_devices=NC)

    pts5c_d = nc.dram_tensor("pts5c", [5 * BL, N], F32, kind="ExternalInput")
    basis5_d = nc.dram_tensor("basis5", [5, P], F32, kind="ExternalInput")
    wts_d = nc.dram_tensor("wts", [128, WCOLS], F16, kind="ExternalInput")
    bias_d = nc.dram_tensor("biases", [1, BIAS_COLS], F16, kind="ExternalInput")
    gpk_d = nc.dram_tensor("gpk", [128, 20], F32, kind="ExternalInput")
    bpk_d = nc.dram_tensor("bpk", [128, 20], F32, kind="ExternalInput")
    outT_d = nc.dram_tensor("outT", [E, B], F32, kind="ExternalOutput")

    cc0_in = nc.dram_tensor("cc0_in", [1, 4], F32)
    cc0_out = nc.dram_tensor("cc0_out", [NC, 4], F32, addr_space="Shared")
    MA = 6  # chunks in the first (overlapped) AllGather
    cc_inA = nc.dram_tensor("cc_inA", [MA * 128, BL], F32)
    cc_outA = nc.dram_tensor("cc_outA", [NC * MA * 128, BL], F32,
                             addr_space="Shared")
    cc_inB = nc.dram_tensor("cc_inB", [(MCH - MA) * 128, BL], F32)
    cc_outB = nc.dram_tensor("cc_outB", [NC * (MCH - MA) * 128, BL], F32,
                             addr_space="Shared")

    with tile.TileContext(nc) as tc:
        with tc.tile_pool(name="sb", bufs=1) as sb:
            # ---- warm up the collectives subsystem ASAP (hides ~40us) ----
            dummy = sb.tile([1, 4], F32)
            nc.gpsimd.memset(dummy[:], 0.0)
            if ABLATE != 2:
                nc.sync.dma_start(cc0_in[:, :], dummy[:])
                nc.gpsimd.collective_compute(
                    "AllGather", mybir.AluOpType.bypass,
                    replica_groups=[list(range(NC))],
                    ins=[cc0_in.ap().opt()], outs=[cc0_out.ap().opt()])

            # ---- inputs to SBUF ----
            pts5rt = sb.tile([128, N], F32)
            basis5rt = sb.tile([128, P], F32)
            for j in range(BL):
                nc.sync.dma_start(pts5rt[32 * j:32 * j + 5, :],
                                  pts5c_d[5 * j:5 * j + 5, :])
                nc.sync.dma_start(basis5rt[32 * j:32 * j + 5, :],
                                  basis5_d[:, :])
            wts = sb.tile([128, WCOLS], F16)
            nc.sync.dma_start(wts[:], wts_d[:])
            biases = sb.tile([1, BIAS_COLS], F16)
            nc.sync.dma_start(biases[:], bias_d[:])
            gpk = sb.tile([128, 20], F32)
            bpk = sb.tile([128, 20], F32)
            nc.sync.dma_start(gpk[:], gpk_d[:])
            nc.sync.dma_start(bpk[:], bpk_d[:])
            ones_h = sb.tile([1, B], F16)
            nc.gpsimd.memset(ones_h[:], 1.0)

            featA = sb.tile([128, MCH, BL], F32)
            featB = sb.tile([128, MCH, BL], F32)

            # ---- BPS: distances + min-reduce ----
            def finalize(m0, m1):
                # combine halves, clamp(0), sqrt for chunks [m0, m1)
                nc.vector.tensor_tensor(featA[:, m0:m1, :], featA[:, m0:m1, :],
                                        featB[:, m0:m1, :], mybir.AluOpType.min)
                nc.vector.tensor_scalar_max(featA[:, m0:m1, :],
                                            featA[:, m0:m1, :], 0.0)
                nc.scalar.activation(featA[:, m0:m1, :], featA[:, m0:m1, :],
                                     mybir.ActivationFunctionType.Sqrt)

            def start_ag(cin, cout, m0, m1):
                nc.sync.dma_start(
                    cin.ap().rearrange("(m p) b -> p m b", p=128),
                    featA[:, m0:m1, :])
                nc.gpsimd.collective_compute(
                    "AllGather", mybir.AluOpType.bypass,
                    replica_groups=[list(range(NC))],
                    ins=[cin.ap().opt()], outs=[cout.ap().opt()])

            if ABLATE == 3:
                nc.gpsimd.memset(featA[:], 1.0)
                nc.gpsimd.memset(featB[:], 1.0)
            with tc.tile_pool(name="psb", bufs=1, space="PSUM") as psb, \
                 tc.tile_pool(name="stg", bufs=2) as stg:
              if ABLATE != 3:
                  gidx = 0
                  for m in range(MCH):
                      for hh in range(2):
                          for j in range(BL):
                            dps = psb.tile([128, 2, 512], F32, tag=f"d{j}",
                                           name=f"d{m}_{hh}_{j}")
                            for t2 in range(2):
                                t = 2 * hh + t2
                                nc.tensor.matmul(
                                    dps[:, t2, :],
                                    basis5rt[32 * j:32 * j + 5,
                                             m * 128:(m + 1) * 128],
                                    pts5rt[32 * j:32 * j + 5,
                                           t * 512:(t + 1) * 512],
                                    start=True, stop=True,
                                    tile_position=(32 * j, 0))
                            dst = featA if hh == 0 else featB
                            if gidx % 8 == 0:
                                # direct DVE reduce from PSUM
                                nc.vector.tensor_reduce(
                                    dst[:, m, j:j + 1], dps[:],
                                    axis=mybir.AxisListType.XY,
                                    op=mybir.AluOpType.min)
                            else:
                                # ACT evacuates PSUM as fp16; DVE min-tree
                                s16 = stg.tile([128, 2, 512], F16,
                                               tag=f"s{j}", name=f"s{gidx}")
                                nc.scalar.activation(
                                    s16[:, :, :], dps[:, :, :],
                                    mybir.ActivationFunctionType.Copy)
                                t16 = stg.tile([128, 512], F16, tag=f"t{j}",
                                               name=f"t{gidx}")
                                nc.vector.tensor_tensor(
                                    t16[:, :], s16[:, 0, :], s16[:, 1, :],
                                    mybir.AluOpType.min)
                                nc.vector.tensor_reduce(
                                    dst[:, m, j:j + 1], t16[:, :],
                                    axis=mybir.AxisListType.X,
                                    op=mybir.AluOpType.min)
                            gidx += 1
                      if m == MA - 1 and ABLATE != 2:
                          finalize(0, MA)
                          start_ag(cc_inA, cc_outA, 0, MA)
                  finalize(MA, MCH)
                  if ABLATE != 2:
                      start_ag(cc_inB, cc_outB, MA, MCH)
              else:
                  finalize(0, MCH)
                  if ABLATE != 2:
                      start_ag(cc_inA, cc_outA, 0, MA)
                      start_ag(cc_inB, cc_outB, MA, MCH)

            # ---- gather feat ----
            feat = sb.tile([128, MCH, NC, BL], F32)
            if ABLATE != 2:
                ccA_r = cc_outA.ap().rearrange("(r m p) b -> p m r b",
                                               p=128, m=MA)
                ccB_r = cc_outB.ap().rearrange("(r m p) b -> p m r b",
                                               p=128, m=MCH - MA)
                for m in range(MCH):
                    if m < MA:
                        nc.sync.dma_start(feat[:, m, :, :], ccA_r[:, m, :, :])
                    else:
                        nc.sync.dma_start(feat[:, m, :, :],
                                          ccB_r[:, m - MA, :, :])
            else:
                for r in range(NC):
                    nc.vector.tensor_copy(feat[:, :, r, :], featA[:, :, :])

            # ================= MLP head (feature-major, batch on free) ======
            # stat columns: bn0:0-7, L0:8-9, L1:10-11, L2:12-13, L3:14-15, LF:16-19
            SCOL = {"bn0": 0, "L0": 8, "L1": 10, "L2": 12, "L3": 14, "LF": 16}
            NT_L = {"bn0": 8, "L0": 2, "L1": 2, "L2": 2, "L3": 2, "LF": 4}
            sums = sb.tile([128, 20], F32)
            sqs = sb.tile([128, 20], F32)
            scr = sb.tile([128, B], F32)
            scr16 = sb.tile([128, B], F16)

            x0 = sb.tile([128, 8, B], F16)
            h1 = sb.tile([128, 2, B], F16)
            a1 = sb.tile([128, 2, B], F16)
            h2 = sb.tile([128, 2, B], F16)
            a2 = sb.tile([128, 2, B], F16)
            outT = sb.tile([128, 4, B], F32)

            def stats(src_ap, lname, i, fp16):
                c = SCOL[lname] + i
                nc.vector.tensor_reduce(
                    sums[:, c:c + 1], src_ap, axis=mybir.AxisListType.X,
                    op=mybir.AluOpType.add)
                sc = (scr16 if fp16 else scr)[:, :]
                nc.vector.tensor_tensor(sc, src_ap, src_ap,
                                        mybir.AluOpType.mult)
                nc.vector.tensor_reduce(
                    sqs[:, c:c + 1], sc, axis=mybir.AxisListType.X,
                    op=mybir.AluOpType.add)

            def bn_coeffs(lname):
                """Turn sums/sqs cols into A (scale) and C (shift) in-place:
                A -> sums cols, C -> sqs cols."""
                c0 = SCOL[lname]
                c1 = c0 + NT_L[lname]
                mean = scr  # reuse scratch [128, B]; only first cols used
                nc.vector.tensor_scalar_mul(mean[:, c0:c1], sums[:, c0:c1],
                                            1.0 / B)
                nc.vector.tensor_scalar_mul(sqs[:, c0:c1], sqs[:, c0:c1],
                                            1.0 / B)
                # var = E[x^2] - mean^2   (into sqs)
                nc.vector.tensor_tensor(scr[:, c1:c1 + NT_L[lname]],
                                        mean[:, c0:c1], mean[:, c0:c1],
                                        mybir.AluOpType.mult)
                nc.vector.tensor_tensor(sqs[:, c0:c1], sqs[:, c0:c1],
                                        scr[:, c1:c1 + NT_L[lname]],
                                        mybir.AluOpType.subtract)
                nc.vector.tensor_scalar_add(sqs[:, c0:c1], sqs[:, c0:c1], EPS)
                nc.vector.reciprocal(sqs[:, c0:c1], sqs[:, c0:c1])
                nc.scalar.activation(sqs[:, c0:c1], sqs[:, c0:c1],
                                     mybir.ActivationFunctionType.Sqrt)
                # A = g * rsqrt  (into sums)
                nc.vector.tensor_tensor(sums[:, c0:c1], gpk[:, c0:c1],
                                        sqs[:, c0:c1], mybir.AluOpType.mult)
                # C = beta - mean * A  (into sqs)
                nc.vector.tensor_tensor(scr[:, c1:c1 + NT_L[lname]],
                                        mean[:, c0:c1], sums[:, c0:c1],
                                        mybir.AluOpType.mult)
                nc.vector.tensor_tensor(sqs[:, c0:c1], bpk[:, c0:c1],
                                        scr[:, c1:c1 + NT_L[lname]],
                                        mybir.AluOpType.subtract)

            def bn_apply(src_ap, dst_ap, lname, i):
                c = SCOL[lname] + i
                nc.vector.tensor_scalar(
                    out=dst_ap, in0=src_ap,
                    scalar1=sums[:, c:c + 1], scalar2=sqs[:, c:c + 1],
                    op0=mybir.AluOpType.mult, op1=mybir.AluOpType.add)

            # ---- bn0 over feat ----
            for m in range(MCH):
                stats(feat[:, m, :, :].rearrange("p r b -> p (r b)"),
                      "bn0", m, False)
            bn_coeffs("bn0")
            for m in range(MCH):
                bn_apply(feat[:, m, :, :].rearrange("p r b -> p (r b)"),
                         x0[:, m, :], "bn0", m)

            wts_off = [0]

            def layer(lname, ktiles, dst, fp32_out=False):
                """ktiles: list of [128, B] fp16 APs (K-tiles of input).
                dst: output tile [128, NT, B]."""
                hout = HOUT[lname]
                nmo = hout // 128
                base = wts_off[0]
                bbase = {"L0": 0, "L1": H, "L2": 2 * H, "L3": 3 * H,
                         "LF": 4 * H}[lname]
                with tc.tile_pool(name=f"pm{lname}", bufs=4,
                                  space="PSUM") as psm:
                    for mo in range(nmo):
                        zp = psm.tile([128, B], F32, tag="z",
                                      name=f"z{lname}_{mo}")
                        nk = len(ktiles)
                        for k, rhs in enumerate(ktiles):
                            lhsT = wts[:, base + k * hout + mo * 128:
                                       base + k * hout + (mo + 1) * 128]
                            nc.tensor.matmul(zp[:, :], lhsT, rhs,
                                             start=(k == 0), stop=False)
                        nc.tensor.matmul(
                            zp[:, :],
                            biases[0:1, bbase + mo * 128:bbase + (mo + 1) * 128],
                            ones_h[0:1, :], start=False, stop=True)
                        # relu -> fp16 (or fp32 for final)
                        if fp32_out:
                            nc.scalar.activation(
                                dst[:, mo, :], zp[:, :],
                                mybir.ActivationFunctionType.Relu)
                            stats(dst[:, mo, :], lname, mo, False)
                        else:
                            nc.scalar.activation(
                                dst[:, mo, :], zp[:, :],
                                mybir.ActivationFunctionType.Relu)
                            stats(dst[:, mo, :], lname, mo, True)
                wts_off[0] = base + NKT[lname] * hout
                bn_coeffs(lname)
                for mo in range(nmo):
                    bn_apply(dst[:, mo, :], dst[:, mo, :], lname, mo)

            if ABLATE == 1:
                nc.vector.memset(outT[:], 0.25)
            else:
                x0k = [x0[:, m, :] for m in range(8)]
                layer("L0", x0k, h1)
                layer("L1", [h1[:, i, :] for i in range(2)], a1)
                layer("L2", x0k + [a1[:, i, :] for i in range(2)], h2)
                layer("L3", [h2[:, i, :] for i in range(2)], a2)
                layer("LF", x0k + [a1[:, i, :] for i in range(2)]
                      + [a2[:, i, :] for i in range(2)], outT, fp32_out=True)

            outT_r = outT_d.ap().rearrange("(mo p) b -> p mo b", p=128)
            nc.sync.dma_start(outT_r[:, :, :], outT[:, :, :])

    nc.compile()
    return nc


def _prep_inputs(x, basis, bn0_g, bn0_b, W0, b0, g0, beta0, W1, b1, g1, beta1,
                 W2, b2, g2, beta2, W3, b3, g3, beta3, Wf, bf, gf, betaf):
    f32 = np.float32
    f16 = np.float16
    x = np.asarray(x, f32)
    s = (x.astype(np.float64) ** 2).sum(1).astype(f32)        # [B, N]
    basis = np.asarray(basis, f32)

    basis5 = np.zeros((5, P), f32)
    basis5[0:3] = -2.0 * basis.T
    basis5[3] = 1.0
    basis5[4] = (basis ** 2).sum(1)

    def ktile_cols(WT, hout):
        # WT: [K, hout] -> [128, nk*hout] column-packed K-tiles
        nk = WT.shape[0] // 128
        return np.concatenate([WT[k * 128:(k + 1) * 128, :]
                               for k in range(nk)], axis=1)

    wts = np.concatenate([
        ktile_cols(np.ascontiguousarray(W0.T), H),
        ktile_cols(np.ascontiguousarray(W1.T), H),
        ktile_cols(np.ascontiguousarray(W2.T), H),
        ktile_cols(np.ascontiguousarray(W3.T), H),
        ktile_cols(np.ascontiguousarray(Wf.T), E),
    ], axis=1).astype(f16)

    biases = np.concatenate([b0, b1, b2, b3, bf]).reshape(1, -1).astype(f16)

    def pk(v, n):
        return np.asarray(v, f32).reshape(n, 128).T

    gpk = np.concatenate([pk(bn0_g, 8), pk(g0, 2), pk(g1, 2), pk(g2, 2),
                          pk(g3, 2), pk(gf, 4)], axis=1)
    bpk = np.concatenate([pk(bn0_b, 8), pk(beta0, 2), pk(beta1, 2),
                          pk(beta2, 2), pk(beta3, 2), pk(betaf, 4)], axis=1)

    in_maps = []
    for c in range(NC):
        pts5c = np.zeros((5 * BL, N), f32)
        for j in range(BL):
            b = c * BL + j
            pts5c[5 * j:5 * j + 3] = x[b]
            pts5c[5 * j + 3] = s[b]
            pts5c[5 * j + 4] = 1.0
        in_maps.append({"pts5c": pts5c, "basis5": basis5, "wts": wts,
                        "biases": biases, "gpk": gpk, "bpk": bpk})
    return in_maps


LAST_EXEC_NS = None
LAST_PROFILE = None


def kernel(**inputs) -> np.ndarray:
    global LAST_EXEC_NS, LAST_PROFILE
    if "nc" not in _CACHE:
        _CACHE["nc"] = _build_module()
    nc = _CACHE["nc"]
    in_maps = _prep_inputs(**inputs)
    res = bass_utils.run_bass_kernel_spmd(
        nc, in_maps, core_ids=list(range(NC)), trace=TRACE)
    LAST_EXEC_NS = res.exec_time_ns
    LAST_PROFILE = res.profile_json
    outT = res.results[0]["outT"]          # [E, B]
    return np.ascontiguousarray(outT.T)    # [B, E]


# revision 10
# speedup vs baseline: 1.2199x; 1.0545x over previous
"""BPS-DenseNet Trainium2 kernel.

kernel(**inputs) -> [32, 512] f32. Shards the BPS distance computation
data-parallel over batch across 8 NeuronCores, AllGathers the BPS features,
then every core computes the (tiny) BN-MLP head redundantly; core 0's output
is returned.
"""
import os
import sys
import types

sys.path.insert(0, '/opt/trn_rl_repo')
import numpy as np

# --- optional NTFF profile hook (only when BPS_TRACE=1; grading path skips) ---
TRACE = os.environ.get("BPS_TRACE", "0") == "1"
ABLATE = int(os.environ.get("BPS_ABLATE", "0"))
if TRACE:
    import antenv
    _mod = types.ModuleType("antenv.axon_hooks")
    _mod._hook = None
    _mod.set_axon_ntff_profile_hook = lambda h: setattr(_mod, "_hook", h)
    _mod.get_axon_ntff_profile_hook = lambda: _mod._hook
    sys.modules["antenv.axon_hooks"] = _mod
    antenv.axon_hooks = _mod
    from trn_agent_boot.trn_boot import _ntff_profile_via_ctypes
    _mod._hook = _ntff_profile_via_ctypes('/opt/axon/libaxon_pjrt.so')

import concourse.bacc as bacc
import concourse.mybir as mybir
import concourse.tile as tile
from concourse import bass_utils

bass_utils.upload_artifacts = lambda tmpdir: tmpdir

B, N, P, H, E = 32, 2048, 1024, 256, 512
NC = 8
BL = B // NC            # batches per core
MCH = P // 128          # basis chunks
EPS = 1e-5

F32 = mybir.dt.float32
F16 = mybir.dt.float16

# layer defs: (name, K-tile count, H_out)
# K-tile sources resolved at emission time.
NKT = {"L0": 8, "L1": 2, "L2": 10, "L3": 2, "LF": 12}
HOUT = {"L0": H, "L1": H, "L2": H, "L3": H, "LF": E}
WCOLS = sum(NKT[l] * HOUT[l] for l in ("L0", "L1", "L2", "L3", "LF"))
BIAS_COLS = 4 * H + E

_CACHE = {}


def _build_module():
    nc = bacc.Bacc("TRN2", target_bir_lowering=False, debug=False,
                   num_devices=NC)

    pts5h_d = nc.dram_tensor("pts5h", [5 * BL, N], F16, kind="ExternalInput")
    pts5l_d = nc.dram_tensor("pts5l", [5 * BL, N], F16, kind="ExternalInput")
    basis5h_d = nc.dram_tensor("basis5h", [5, P], F16, kind="ExternalInput")
    basis5l_d = nc.dram_tensor("basis5l", [5, P], F16, kind="ExternalInput")
    wts_d = nc.dram_tensor("wts", [128, WCOLS], F16, kind="ExternalInput")
    bias_d = nc.dram_tensor("biases", [1, BIAS_COLS], F16, kind="ExternalInput")
    gpk_d = nc.dram_tensor("gpk", [128, 20], F32, kind="ExternalInput")
    bpk_d = nc.dram_tensor("bpk", [128, 20], F32, kind="ExternalInput")
    outT_d = nc.dram_tensor("outT", [E, B], F32, kind="ExternalOutput")

    cc0_in = nc.dram_tensor("cc0_in", [1, 4], F32)
    cc0_out = nc.dram_tensor("cc0_out", [NC, 4], F32, addr_space="Shared")
    MA = 6  # chunks in the first (overlapped) AllGather
    cc_inA = nc.dram_tensor("cc_inA", [MA * 128, BL], F32)
    cc_outA = nc.dram_tensor("cc_outA", [NC * MA * 128, BL], F32,
                             addr_space="Shared")
    cc_inB = nc.dram_tensor("cc_inB", [(MCH - MA) * 128, BL], F32)
    cc_outB = nc.dram_tensor("cc_outB", [NC * (MCH - MA) * 128, BL], F32,
                             addr_space="Shared")

    with tile.TileContext(nc) as tc:
        with tc.tile_pool(name="sb", bufs=1) as sb:
            # ---- warm up the collectives subsystem ASAP (hides ~40us) ----
            dummy = sb.tile([1, 4], F32)
            nc.gpsimd.memset(dummy[:], 0.0)
            if ABLATE != 2:
                nc.sync.dma_start(cc0_in[:, :], dummy[:])
                nc.gpsimd.collective_compute(
                    "AllGather", mybir.AluOpType.bypass,
                    replica_groups=[list(range(NC))],
                    ins=[cc0_in.ap().opt()], outs=[cc0_out.ap().opt()])

            # ---- inputs to SBUF ----
            pts5h = sb.tile([128, N], F16)
            pts5l = sb.tile([128, N], F16)
            basis5h = sb.tile([128, P], F16)
            basis5l = sb.tile([128, P], F16)
            for j in range(BL):
                nc.sync.dma_start(pts5h[32 * j:32 * j + 5, :],
                                  pts5h_d[5 * j:5 * j + 5, :])
                nc.sync.dma_start(pts5l[32 * j:32 * j + 5, :],
                                  pts5l_d[5 * j:5 * j + 5, :])
                nc.sync.dma_start(basis5h[32 * j:32 * j + 5, :],
                                  basis5h_d[:, :])
                nc.sync.dma_start(basis5l[32 * j:32 * j + 5, :],
                                  basis5l_d[:, :])
            wts = sb.tile([128, WCOLS], F16)
            nc.sync.dma_start(wts[:], wts_d[:])
            biases = sb.tile([1, BIAS_COLS], F16)
            nc.sync.dma_start(biases[:], bias_d[:])
            gpk = sb.tile([128, 20], F32)
            bpk = sb.tile([128, 20], F32)
            nc.sync.dma_start(gpk[:], gpk_d[:])
            nc.sync.dma_start(bpk[:], bpk_d[:])
            ones_h = sb.tile([1, B], F16)
            nc.gpsimd.memset(ones_h[:], 1.0)

            featA = sb.tile([128, MCH, BL], F32)
            featB = sb.tile([128, MCH, BL], F32)

            # ---- BPS: distances + min-reduce ----
            def finalize(m0, m1):
                # combine halves, clamp(0), sqrt for chunks [m0, m1)
                nc.vector.tensor_tensor(featA[:, m0:m1, :], featA[:, m0:m1, :],
                                        featB[:, m0:m1, :], mybir.AluOpType.min)
                nc.vector.tensor_scalar_max(featA[:, m0:m1, :],
                                            featA[:, m0:m1, :], 0.0)
                nc.scalar.activation(featA[:, m0:m1, :], featA[:, m0:m1, :],
                                     mybir.ActivationFunctionType.Sqrt)

            def start_ag(cin, cout, m0, m1):
                nc.sync.dma_start(
                    cin.ap().rearrange("(m p) b -> p m b", p=128),
                    featA[:, m0:m1, :])
                nc.gpsimd.collective_compute(
                    "AllGather", mybir.AluOpType.bypass,
                    replica_groups=[list(range(NC))],
                    ins=[cin.ap().opt()], outs=[cout.ap().opt()])

            if ABLATE == 3:
                nc.gpsimd.memset(featA[:], 1.0)
                nc.gpsimd.memset(featB[:], 1.0)
            with tc.tile_pool(name="psb", bufs=1, space="PSUM") as psb, \
                 tc.tile_pool(name="stg", bufs=2) as stg:
              if ABLATE != 3:
                  gidx = 0
                  for m in range(MCH):
                      for hh in range(2):
                          for j in range(BL):
                            dps = psb.tile([128, 2, 512], F32, tag=f"d{j}",
                                           name=f"d{m}_{hh}_{j}")
                            for t2 in range(2):
                                t = 2 * hh + t2
                                bh = basis5h[32 * j:32 * j + 5,
                                             m * 128:(m + 1) * 128]
                                bl = basis5l[32 * j:32 * j + 5,
                                             m * 128:(m + 1) * 128]
                                ph = pts5h[32 * j:32 * j + 5,
                                           t * 512:(t + 1) * 512]
                                pl = pts5l[32 * j:32 * j + 5,
                                           t * 512:(t + 1) * 512]
                                for lhs, rhs_, st, sp in (
                                        (bh, ph, True, False),
                                        (bh, pl, False, False),
                                        (bl, ph, False, True)):
                                    nc.tensor.matmul(
                                        dps[:, t2, :], lhs, rhs_,
                                        start=st, stop=sp,
                                        tile_position=(32 * j, 0))
                            dst = featA if hh == 0 else featB
                            if gidx % 8 == 0:
                                # direct DVE reduce from PSUM
                                nc.vector.tensor_reduce(
                                    dst[:, m, j:j + 1], dps[:],
                                    axis=mybir.AxisListType.XY,
                                    op=mybir.AluOpType.min)
                            else:
                                # ACT evacuates PSUM as fp16; DVE min-tree
                                s16 = stg.tile([128, 2, 512], F16,
                                               tag=f"s{j}", name=f"s{gidx}")
                                nc.scalar.activation(
                                    s16[:, :, :], dps[:, :, :],
                                    mybir.ActivationFunctionType.Copy)
                                t16 = stg.tile([128, 512], F16, tag=f"t{j}",
                                               name=f"t{gidx}")
                                nc.vector.tensor_tensor(
                                    t16[:, :], s16[:, 0, :], s16[:, 1, :],
                                    mybir.AluOpType.min)
                                nc.vector.tensor_reduce(
                                    dst[:, m, j:j + 1], t16[:, :],
                                    axis=mybir.AxisListType.X,
                                    op=mybir.AluOpType.min)
                            gidx += 1
                      if m == MA - 1 and ABLATE != 2:
                          finalize(0, MA)
                          start_ag(cc_inA, cc_outA, 0, MA)
                  finalize(MA, MCH)
                  if ABLATE != 2:
                      start_ag(cc_inB, cc_outB, MA, MCH)
              else:
                  finalize(0, MCH)
                  if ABLATE != 2:
                      start_ag(cc_inA, cc_outA, 0, MA)
                      start_ag(cc_inB, cc_outB, MA, MCH)

            # ---- gather feat ----
            feat = sb.tile([128, MCH, NC, BL], F32)
            if ABLATE != 2:
                ccA_r = cc_outA.ap().rearrange("(r m p) b -> p m r b",
                                               p=128, m=MA)
                ccB_r = cc_outB.ap().rearrange("(r m p) b -> p m r b",
                                               p=128, m=MCH - MA)
                for m in range(MCH):
                    if m < MA:
                        nc.sync.dma_start(feat[:, m, :, :], ccA_r[:, m, :, :])
                    else:
                        nc.sync.dma_start(feat[:, m, :, :],
                                          ccB_r[:, m - MA, :, :])
            else:
                for r in range(NC):
                    nc.vector.tensor_copy(feat[:, :, r, :], featA[:, :, :])

            # ================= MLP head (feature-major, batch on free) ======
            # stat columns: bn0:0-7, L0:8-9, L1:10-11, L2:12-13, L3:14-15, LF:16-19
            SCOL = {"bn0": 0, "L0": 8, "L1": 10, "L2": 12, "L3": 14, "LF": 16}
            NT_L = {"bn0": 8, "L0": 2, "L1": 2, "L2": 2, "L3": 2, "LF": 4}
            sums = sb.tile([128, 20], F32)
            sqs = sb.tile([128, 20], F32)
            scr = sb.tile([128, B], F32)
            scr16 = sb.tile([128, B], F16)

            x0 = sb.tile([128, 8, B], F16)
            h1 = sb.tile([128, 2, B], F16)
            a1 = sb.tile([128, 2, B], F16)
            h2 = sb.tile([128, 2, B], F16)
            a2 = sb.tile([128, 2, B], F16)
            outT = sb.tile([128, 4, B], F32)

            def stats(src_ap, lname, i, fp16):
                c = SCOL[lname] + i
                nc.vector.tensor_reduce(
                    sums[:, c:c + 1], src_ap, axis=mybir.AxisListType.X,
                    op=mybir.AluOpType.add)
                sc = (scr16 if fp16 else scr)[:, :]
                nc.vector.tensor_tensor(sc, src_ap, src_ap,
                                        mybir.AluOpType.mult)
                nc.vector.tensor_reduce(
                    sqs[:, c:c + 1], sc, axis=mybir.AxisListType.X,
                    op=mybir.AluOpType.add)

            def bn_coeffs(lname, lo=0, hi=None):
                """Turn sums/sqs cols into A (scale) and C (shift) in-place:
                A -> sums cols, C -> sqs cols."""
                c0 = SCOL[lname] + lo
                c1 = SCOL[lname] + (NT_L[lname] if hi is None else hi)
                mean = scr  # reuse scratch [128, B]; only first cols used
                nc.vector.tensor_scalar_mul(mean[:, c0:c1], sums[:, c0:c1],
                                            1.0 / B)
                nc.vector.tensor_scalar_mul(sqs[:, c0:c1], sqs[:, c0:c1],
                                            1.0 / B)
                # var = E[x^2] - mean^2   (into sqs)
                nc.vector.tensor_tensor(scr[:, c1:c1 + (c1 - c0)],
                                        mean[:, c0:c1], mean[:, c0:c1],
                                        mybir.AluOpType.mult)
                nc.vector.tensor_tensor(sqs[:, c0:c1], sqs[:, c0:c1],
                                        scr[:, c1:c1 + (c1 - c0)],
                                        mybir.AluOpType.subtract)
                nc.vector.tensor_scalar_add(sqs[:, c0:c1], sqs[:, c0:c1], EPS)
                nc.vector.reciprocal(sqs[:, c0:c1], sqs[:, c0:c1])
                nc.scalar.activation(sqs[:, c0:c1], sqs[:, c0:c1],
                                     mybir.ActivationFunctionType.Sqrt)
                # A = g * rsqrt  (into sums)
                nc.vector.tensor_tensor(sums[:, c0:c1], gpk[:, c0:c1],
                                        sqs[:, c0:c1], mybir.AluOpType.mult)
                # C = beta - mean * A  (into sqs)
                nc.vector.tensor_tensor(scr[:, c1:c1 + (c1 - c0)],
                                        mean[:, c0:c1], sums[:, c0:c1],
                                        mybir.AluOpType.mult)
                nc.vector.tensor_tensor(sqs[:, c0:c1], bpk[:, c0:c1],
                                        scr[:, c1:c1 + (c1 - c0)],
                                        mybir.AluOpType.subtract)

            def bn_apply(src_ap, dst_ap, lname, i):
                c = SCOL[lname] + i
                nc.vector.tensor_scalar(
                    out=dst_ap, in0=src_ap,
                    scalar1=sums[:, c:c + 1], scalar2=sqs[:, c:c + 1],
                    op0=mybir.AluOpType.mult, op1=mybir.AluOpType.add)

            # ---- bn0 over feat (split: first MA chunks can start under AG-B)
            for m in range(MA):
                stats(feat[:, m, :, :].rearrange("p r b -> p (r b)"),
                      "bn0", m, False)
            bn_coeffs("bn0", 0, MA)
            for m in range(MA):
                bn_apply(feat[:, m, :, :].rearrange("p r b -> p (r b)"),
                         x0[:, m, :], "bn0", m)
            for m in range(MA, MCH):
                stats(feat[:, m, :, :].rearrange("p r b -> p (r b)"),
                      "bn0", m, False)
            bn_coeffs("bn0", MA, MCH)
            for m in range(MA, MCH):
                bn_apply(feat[:, m, :, :].rearrange("p r b -> p (r b)"),
                         x0[:, m, :], "bn0", m)

            wts_off = [0]

            def layer(lname, ktiles, dst, fp32_out=False):
                """ktiles: list of [128, B] fp16 APs (K-tiles of input).
                dst: output tile [128, NT, B]."""
                hout = HOUT[lname]
                nmo = hout // 128
                base = wts_off[0]
                bbase = {"L0": 0, "L1": H, "L2": 2 * H, "L3": 3 * H,
                         "LF": 4 * H}[lname]
                with tc.tile_pool(name=f"pm{lname}", bufs=4,
                                  space="PSUM") as psm:
                    for mo in range(nmo):
                        zp = psm.tile([128, B], F32, tag="z",
                                      name=f"z{lname}_{mo}")
                        nk = len(ktiles)
                        for k, rhs in enumerate(ktiles):
                            lhsT = wts[:, base + k * hout + mo * 128:
                                       base + k * hout + (mo + 1) * 128]
                            nc.tensor.matmul(zp[:, :], lhsT, rhs,
                                             start=(k == 0), stop=False)
                        nc.tensor.matmul(
                            zp[:, :],
                            biases[0:1, bbase + mo * 128:bbase + (mo + 1) * 128],
                            ones_h[0:1, :], start=False, stop=True)
                        # relu -> fp16 (or fp32 for final)
                        if fp32_out:
                            nc.scalar.activation(
                                dst[:, mo, :], zp[:, :],
                                mybir.ActivationFunctionType.Relu)
                            stats(dst[:, mo, :], lname, mo, False)
                        else:
                            nc.scalar.activation(
                                dst[:, mo, :], zp[:, :],
                                mybir.ActivationFunctionType.Relu)
                            stats(dst[:, mo, :], lname, mo, True)
                wts_off[0] = base + NKT[lname] * hout
                bn_coeffs(lname)
                for mo in range(nmo):
                    bn_apply(dst[:, mo, :], dst[:, mo, :], lname, mo)

            if ABLATE == 1:
                nc.vector.memset(outT[:], 0.25)
            else:
                x0k = [x0[:, m, :] for m in range(8)]
                layer("L0", x0k, h1)
                layer("L1", [h1[:, i, :] for i in range(2)], a1)
                layer("L2", x0k + [a1[:, i, :] for i in range(2)], h2)
                layer("L3", [h2[:, i, :] for i in range(2)], a2)
                layer("LF", x0k + [a1[:, i, :] for i in range(2)]
                      + [a2[:, i, :] for i in range(2)], outT, fp32_out=True)

            outT_r = outT_d.ap().rearrange("(mo p) b -> p mo b", p=128)
            nc.sync.dma_start(outT_r[:, :, :], outT[:, :, :])

    nc.compile()
    return nc


def _prep_inputs(x, basis, bn0_g, bn0_b, W0, b0, g0, beta0, W1, b1, g1, beta1,
                 W2, b2, g2, beta2, W3, b3, g3, beta3, Wf, bf, gf, betaf):
    f32 = np.float32
    f16 = np.float16
    x = np.asarray(x, f32)
    s = (x.astype(np.float64) ** 2).sum(1).astype(f32)        # [B, N]
    basis = np.asarray(basis, f32)

    basis5 = np.zeros((5, P), f32)
    basis5[0:3] = -2.0 * basis.T
    basis5[3] = 1.0
    basis5[4] = (basis ** 2).sum(1)
    basis5h = basis5.astype(f16)
    basis5l = (basis5 - basis5h.astype(f32)).astype(f16)

    def ktile_cols(WT, hout):
        # WT: [K, hout] -> [128, nk*hout] column-packed K-tiles
        nk = WT.shape[0] // 128
        return np.concatenate([WT[k * 128:(k + 1) * 128, :]
                               for k in range(nk)], axis=1)

    wts = np.concatenate([
        ktile_cols(np.ascontiguousarray(W0.T), H),
        ktile_cols(np.ascontiguousarray(W1.T), H),
        ktile_cols(np.ascontiguousarray(W2.T), H),
        ktile_cols(np.ascontiguousarray(W3.T), H),
        ktile_cols(np.ascontiguousarray(Wf.T), E),
    ], axis=1).astype(f16)

    biases = np.concatenate([b0, b1, b2, b3, bf]).reshape(1, -1).astype(f16)

    def pk(v, n):
        return np.asarray(v, f32).reshape(n, 128).T

    gpk = np.concatenate([pk(bn0_g, 8), pk(g0, 2), pk(g1, 2), pk(g2, 2),
                          pk(g3, 2), pk(gf, 4)], axis=1)
    bpk = np.concatenate([pk(bn0_b, 8), pk(beta0, 2), pk(beta1, 2),
                          pk(beta2, 2), pk(beta3, 2), pk(betaf, 4)], axis=1)

    in_maps = []
    for c in range(NC):
        pts5c = np.zeros((5 * BL, N), f32)
        for j in range(BL):
            b = c * BL + j
            pts5c[5 * j:5 * j + 3] = x[b]
            pts5c[5 * j + 3] = s[b]
            pts5c[5 * j + 4] = 1.0
        p5h = pts5c.astype(f16)
        p5l = (pts5c - p5h.astype(f32)).astype(f16)
        in_maps.append({"pts5h": p5h, "pts5l": p5l, "basis5h": basis5h,
                        "basis5l": basis5l, "wts": wts,
                        "biases": biases, "gpk": gpk, "bpk": bpk})
    return in_maps


LAST_EXEC_NS = None
LAST_PROFILE = None


def kernel(**inputs) -> np.ndarray:
    global LAST_EXEC_NS, LAST_PROFILE
    if "nc" not in _CACHE:
        _CACHE["nc"] = _build_module()
    nc = _CACHE["nc"]
    in_maps = _prep_inputs(**inputs)
    res = bass_utils.run_bass_kernel_spmd(
        nc, in_maps, core_ids=list(range(NC)), trace=TRACE)
    LAST_EXEC_NS = res.exec_time_ns
    LAST_PROFILE = res.profile_json
    outT = res.results[0]["outT"]          # [E, B]
    return np.ascontiguousarray(outT.T)    # [B, E]
